# revision 1
# baseline (speedup 1.0000x reference)
"""Inverse DWT (BackwardTransformLayer) Trainium2 Bass kernel.

Math (polyphase form of the zero-interleaved circular FFT convolution):
  out[r, 2p+pi] = sum_{s=0..3} cD[pi,s]*D[r,(p-s)%M] + cA[pi,s]*A[r,(p-s)%M]
  cD[0,s] = w[7-2s]   cD[1,s] = w[6-2s]   cA[0,s] = w[2s]   cA[1,s] = -w[2s+1]

Sharding: data-parallel over rows; 512 rows per core on 8 NeuronCores
(run_bass_kernel_spmd). All compute is exact fp32.

Per core, each 128-row tile is loaded once into SBUF with a 3-column
circular prefix; the 32 (row-tile, parity, 1024-col panel) work units are
split between VectorE (21 units: scalar_tensor_tensor MAC chains, first
product on ScalarE, final MAC written stride-2 into the interleaved output
tile) and TensorE (11 units: 8 identity-scaled fp32 matmuls accumulating in
a PSUM bank per 512-col chunk, evicted stride-2 by ScalarE). GPSIMD is left
idle on purpose: its SBUF port is shared with VectorE, so 2-input gpsimd
ops serialize against the DVE instead of adding throughput (measured).

Measured on trn2 (in-kernel repetition method): ~165 us/core vs ~82 us
pure-IO floor (~33.5 MB/core HBM traffic); fp32 MAC throughput is the
binding constraint (DVE scalar_tensor_tensor runs at 1 elem/lane/cycle;
fp32 matmul at 4 cycles/row). float32r (tf32-like) matmuls would reach
~95-120 us but introduce ~1.6e-4 relative error (DWT_F32R=1 enables this
variant); exact fp32 is the default. Relative error vs the FFT reference:
2.9e-07.
"""

import os
import sys

import numpy as np

for _p in ("/opt/trn_rl_repo", "/root/.axon_site/_ro/trn_rl_repo"):
    if os.path.isdir(_p) and _p not in sys.path:
        sys.path.append(_p)

import concourse.bass as bass  # noqa: E402
import concourse.tile as tile  # noqa: E402
from concourse import bacc, mybir  # noqa: E402
from concourse.bass_utils import run_bass_kernel_spmd  # noqa: E402

F32 = mybir.dt.float32
F32R = mybir.dt.float32r
COPY = mybir.ActivationFunctionType.Copy
MUL = mybir.AluOpType.mult
ADD = mybir.AluOpType.add

N_CORES = 8
P = 128          # partitions
M = 4096         # input row length
ROWS = 512       # rows per core
NT = ROWS // P   # row tiles per core
WU = int(os.environ.get("DWT_WU", "1024"))
NPAN = M // WU   # panels per parity
NCHUNK = 512     # psum chunk (one bank of fp32)
INP_BUFS = int(os.environ.get("DWT_INP_BUFS", "2"))
OUT_BUFS = int(os.environ.get("DWT_OUT_BUFS", "3"))
ACC_BUFS = int(os.environ.get("DWT_ACC_BUFS", "8"))
GTMP_BUFS = int(os.environ.get("DWT_GTMP_BUFS", "4"))
PSUM_BUFS = int(os.environ.get("DWT_PSUM_BUFS", "8"))
DVE_SELF_PROD = bool(int(os.environ.get("DWT_DVE_SELF_PROD", "0")))
SPLIT_CHAINS = bool(int(os.environ.get("DWT_SPLIT_CHAINS", "0")))
REPS = int(os.environ.get("DWT_REPS", "1"))  # benchmark-only: repeat body in-kernel
IO_ONLY = bool(int(os.environ.get("DWT_IO_ONLY", "0")))
F32R_MODE = bool(int(os.environ.get("DWT_F32R", "0")))


def _unit_plan():
    """Greedy engine assignment for the NT*2*NPAN work units."""
    force = os.environ.get("DWT_FORCE_ENG")
    if force:
        return {(t, pi, p): force for t in range(NT) for p in range(NPAN) for pi in range(2)}
    counts = os.environ.get("DWT_PLAN", "DVE:21,PE:11,GPS:0")  # e.g. "DVE:16,PE:9,GPS:7"
    if counts:
        quota = {}
        for part in counts.split(","):
            e, n = part.split(":")
            quota[e] = int(n)
        assert sum(quota.values()) == NT * 2 * NPAN, quota
        cost = {"PE": 16.4, "DVE": 8.6, "GPS": 14.9}
        load = {e: 0.0 for e in quota}
        plan = {}
        for t in range(NT):
            for p in range(NPAN):
                for pi in range(2):
                    avail = [e for e in quota if quota[e] > 0]
                    eng = min(avail, key=lambda e: load[e] + cost[e])
                    quota[eng] -= 1
                    load[eng] += cost[eng]
                    plan[(t, pi, p)] = eng
        return plan
    cost = {
        "PE": float(os.environ.get("DWT_COST_PE", "16.4")) * WU / 1024,
        "DVE": float(os.environ.get("DWT_COST_DVE", "8.1")) * WU / 1024,
        "GPS": float(os.environ.get("DWT_COST_GPS", "14.9")) * WU / 1024,
    }
    load = {"PE": 0.0, "DVE": 0.0, "GPS": 0.0}
    plan = {}
    for t in range(NT):
        for p in range(NPAN):
            for pi in range(2):
                eng = min(cost, key=lambda e: load[e] + cost[e])
                load[eng] += cost[eng]
                plan[(t, pi, p)] = eng
    return plan


def build_nc(plan):
    nc = bacc.Bacc()
    det = nc.declare_dram_parameter("details", [ROWS, M], F32, isOutput=False)
    app = nc.declare_dram_parameter("approximation", [ROWS, M], F32, isOutput=False)
    wav = nc.declare_dram_parameter("wavelet", [8], F32, isOutput=False)
    res = nc.declare_dram_parameter("result", [ROWS, 2 * M], F32, isOutput=True)
    ident = nc.inline_tensor(np.eye(P, dtype=np.float32), "ident")

    with tile.TileContext(nc) as tc:
        with (
            tc.tile_pool(name="const", bufs=1) as constp,
            tc.tile_pool(name="ine", bufs=INP_BUFS) as inp,
            tc.tile_pool(name="oute", bufs=OUT_BUFS) as outp,
            tc.tile_pool(name="acc", bufs=ACC_BUFS) as accp,
            tc.tile_pool(name="gtmp", bufs=GTMP_BUFS) as gtmpp,
            tc.tile_pool(name="psum", bufs=PSUM_BUFS, space="PSUM") as psump,
        ):
            # ---- coefficients: broadcast wavelet to all partitions
            wv = constp.tile([1, 8], F32)
            nc.sync.dma_start(wv[:], wav[None, :])
            wvb = constp.tile([P, 8], F32)
            nc.gpsimd.partition_broadcast(wvb[:], wv[:])
            wvn = constp.tile([P, 8], F32)
            nc.vector.tensor_scalar(wvn[:], wvb[:], -1.0, None, MUL)

            def coeff(x, pi, s):
                # x: 0 = details, 1 = approximation; pi: 0 = even, 1 = odd
                if x == 0:
                    idx = (7 - 2 * s) if pi == 0 else (6 - 2 * s)
                    return wvb[:, idx : idx + 1]
                if pi == 0:
                    idx = 2 * s
                    return wvb[:, idx : idx + 1]
                idx = 2 * s + 1
                return wvn[:, idx : idx + 1]

            # ---- PE weights: c * I for each (input, parity, tap)
            it = constp.tile([P, P], F32)
            nc.sync.dma_start(it[:], ident[:, :])
            w16 = constp.tile([P, 16 * P], F32R if F32R_MODE else F32)

            def wslice(x, pi, s):
                j = (x * 2 + pi) * 4 + s
                return w16[:, j * P : (j + 1) * P]

            for x in range(2):
                for pi in range(2):
                    for s in range(4):
                        nc.vector.tensor_scalar(
                            wslice(x, pi, s), it[:], coeff(x, pi, s), None, MUL
                        )

            taps = [(x, s) for x in range(2) for s in range(4)]

            def body(_i=None):
              for t in range(NT):
                  r0 = t * P
                  dt_ext = F32R if F32R_MODE else F32
                  ld = nc.gpsimd.dma_start if F32R_MODE else nc.sync.dma_start
                  dext = inp.tile([P, M + 3], dt_ext, tag="dext")
                  ld(dext[:, 3 : M + 3], det[r0 : r0 + P, :])
                  aext = inp.tile([P, M + 3], dt_ext, tag="aext")
                  ld(aext[:, 3 : M + 3], app[r0 : r0 + P, :])
                  if F32R_MODE:
                      nc.vector.tensor_copy(dext[:, 0:3], dext[:, M : M + 3])
                      nc.vector.tensor_copy(aext[:, 0:3], aext[:, M : M + 3])
                  else:
                      nc.scalar.copy(dext[:, 0:3], dext[:, M : M + 3])
                      nc.scalar.copy(aext[:, 0:3], aext[:, M : M + 3])
                  ext = [dext, aext]

                  def xv(x, a, b):
                      v = ext[x][:, a:b]
                      return v.bitcast(F32) if F32R_MODE else v

                  for h in range(2):  # two output half-tiles of M cols each
                      oh = outp.tile([P, M], F32, tag="out")
                      for p in range(h * (NPAN // 2), (h + 1) * (NPAN // 2)):
                          c0 = p * WU
                          for pi in range(2):
                              eng = plan[(t, pi, p)]
                              base = 2 * c0 - h * M + pi
                              oview = oh[:, base : min(base + 2 * WU, M) : 2]
                              if IO_ONLY:
                                  nc.scalar.copy(oview, xv(0, 3 + c0, 3 + c0 + WU))
                                  continue
                              if eng == "PE":
                                  ccs = list(range(c0, c0 + WU, NCHUNK))
                                  pss = [psump.tile([P, NCHUNK], F32, tag="ps", name=f"ps_{t}_{pi}_{p}_{ci2}") for ci2 in range(len(ccs))]
                                  for j, (x, s) in enumerate(taps):
                                      w = wslice(x, pi, s)
                                      for ci, cc in enumerate(ccs):
                                          rhs = ext[x][:, 3 - s + cc : 3 - s + cc + NCHUNK]
                                          nc.tensor.matmul(
                                              pss[ci][:], w, rhs,
                                              start=(j == 0), stop=(j == len(taps) - 1),
                                          )
                                  for ci, cc in enumerate(ccs):
                                      evb = 2 * cc - h * M + pi
                                      nc.scalar.copy(
                                          oh[:, evb : min(evb + 2 * NCHUNK, M) : 2], pss[ci][:]
                                      )
                              elif eng == "DVE":
                                  if SPLIT_CHAINS:
                                      accD = accp.tile([P, WU], F32, tag="acc")
                                      accA = accp.tile([P, WU], F32, tag="acc")
                                      for x, acc in ((0, accD), (1, accA)):
                                          nc.vector.tensor_scalar(
                                              acc[:], xv(x, 3 + c0, 3 + c0 + WU),
                                              coeff(x, pi, 0), None, MUL,
                                          )
                                          for s in range(1, 4):
                                              nc.vector.scalar_tensor_tensor(
                                                  acc[:],
                                                  xv(x, 3 - s + c0, 3 - s + c0 + WU),
                                                  coeff(x, pi, s), acc[:], MUL, ADD,
                                              )
                                      nc.vector.tensor_tensor(oview, accD[:], accA[:], ADD)
                                  else:
                                      acc = accp.tile([P, WU], F32, tag="acc")
                                      if DVE_SELF_PROD:
                                          nc.vector.tensor_scalar(
                                              acc[:], xv(0, 3 + c0, 3 + c0 + WU),
                                              coeff(0, pi, 0), None, MUL,
                                          )
                                      else:
                                          nc.scalar.activation(
                                              acc[:], xv(0, 3 + c0, 3 + c0 + WU),
                                              COPY, scale=coeff(0, pi, 0),
                                          )
                                      for x, s in taps[1:-1]:
                                          nc.vector.scalar_tensor_tensor(
                                              acc[:],
                                              ext[x][:, 3 - s + c0 : 3 - s + c0 + WU],
                                              coeff(x, pi, s), acc[:], MUL, ADD,
                                          )
                                      x, s = taps[-1]
                                      nc.vector.scalar_tensor_tensor(
                                          oview,
                                          xv(x, 3 - s + c0, 3 - s + c0 + WU),
                                          coeff(x, pi, s), acc[:], MUL, ADD,
                                      )
                              else:  # GPS
                                  if SPLIT_CHAINS:
                                      accD = accp.tile([P, WU], F32, tag="acc")
                                      accA = accp.tile([P, WU], F32, tag="acc")
                                      for x, acc in ((0, accD), (1, accA)):
                                          nc.scalar.activation(
                                              acc[:], xv(x, 3 + c0, 3 + c0 + WU),
                                              COPY, scale=coeff(x, pi, 0),
                                          )
                                          for s in range(1, 4):
                                              tmp = gtmpp.tile([P, WU], F32, tag="gtmp")
                                              nc.scalar.activation(
                                                  tmp[:],
                                                  xv(x, 3 - s + c0, 3 - s + c0 + WU),
                                                  COPY, scale=coeff(x, pi, s),
                                              )
                                              nc.gpsimd.tensor_tensor(acc[:], acc[:], tmp[:], ADD)
                                      nc.gpsimd.tensor_tensor(oview, accD[:], accA[:], ADD)
                                  else:
                                      acc = accp.tile([P, WU], F32, tag="acc")
                                      nc.scalar.activation(
                                          acc[:], ext[0][:, 3 + c0 : 3 + c0 + WU],
                                          COPY, scale=coeff(0, pi, 0),
                                      )
                                      for x, s in taps[1:]:
                                          tmp = gtmpp.tile([P, WU], F32, tag="gtmp")
                                          nc.scalar.activation(
                                              tmp[:],
                                              xv(x, 3 - s + c0, 3 - s + c0 + WU),
                                              COPY, scale=coeff(x, pi, s),
                                          )
                                          nc.gpsimd.tensor_tensor(acc[:], acc[:], tmp[:], ADD)
                                      nc.scalar.copy(oview, acc[:])
                      nc.sync.dma_start(res[r0 : r0 + P, h * M : (h + 1) * M], oh[:])

            if REPS == 1:
                body()
            else:
                with tc.For_i(0, REPS, 1) as _rv:
                    body(_rv)
    nc.finalize()
    return nc


_CACHE = {}


def _get_nc():
    if "nc" not in _CACHE:
        _CACHE["nc"] = build_nc(_unit_plan())
    return _CACHE["nc"]


def kernel(details, approximation, wavelet):
    details = np.ascontiguousarray(np.asarray(details, dtype=np.float32))
    approximation = np.ascontiguousarray(np.asarray(approximation, dtype=np.float32))
    wavelet = np.ascontiguousarray(np.asarray(wavelet, dtype=np.float32))
    assert details.shape == (N_CORES * ROWS, M) and approximation.shape == details.shape
    assert wavelet.shape == (8,)

    in_maps = [
        {
            "details": details[c * ROWS : (c + 1) * ROWS],
            "approximation": approximation[c * ROWS : (c + 1) * ROWS],
            "wavelet": wavelet,
        }
        for c in range(N_CORES)
    ]
    trace = bool(int(os.environ.get("DWT_TRACE", "0")))
    r = run_bass_kernel_spmd(_get_nc(), in_maps, list(range(N_CORES)), trace=trace)
    _CACHE["last_results"] = r
    return np.concatenate([r.results[c]["result"] for c in range(N_CORES)], axis=0)



# revision 2
# speedup vs baseline: 1.1181x; 1.1181x over previous
"""Inverse DWT (BackwardTransformLayer) Trainium2 Bass kernel — v2.

Math (polyphase form of the zero-interleaved circular FFT convolution):
  out[r, 2p+pi] = sum_{s=0..3} cD[pi,s]*D[r,(p-s)%M] + cA[pi,s]*A[r,(p-s)%M]
  cD[0,s] = w[7-2s]   cD[1,s] = w[6-2s]   cA[0,s] = w[2s]   cA[1,s] = -w[2s+1]

Sharding: data-parallel over rows; 512 rows per core on 8 NeuronCores.

v2 strategy (vs the all-fp32 v1 at ~210us): the correctness gate is loose
(rel 2e-2), so compute drops to 16-bit where it doubles throughput:
  - PE panels: f32r (tf32-like) matmuls of identity-scaled weights reading
    the fp32 input tiles directly via bitcast — 1 cycle/row instead of
    fp32's 4, no conversion pass needed.
  - DVE panels: fp16 scalar_tensor_tensor MAC chains in 2x_1P packed mode
    (2 elem/lane/cycle). Shifted windows must stay 4B-aligned for 2x, so
    ScalarE converts each panel twice from the fp32 tile: bufE at ext[c0]
    and bufO at ext[c0+1]; taps s in {1,3} read bufE, s in {0,2} read bufO,
    always at even element offsets.
All engines then sit under the ~85us/core HBM floor (33.5 MB/core at
~400 GB/s), making the kernel DMA-bound.
"""

import os
import sys

import numpy as np

for _p in ("/opt/trn_rl_repo", "/root/.axon_site/_ro/trn_rl_repo"):
    if os.path.isdir(_p) and _p not in sys.path:
        sys.path.append(_p)

import concourse.bass as bass  # noqa: E402
import concourse.tile as tile  # noqa: E402
from concourse import bacc, mybir  # noqa: E402
from concourse.bass_utils import run_bass_kernel_spmd  # noqa: E402

F32 = mybir.dt.float32
F32R = mybir.dt.float32r
F16 = mybir.dt.float16
COPY = mybir.ActivationFunctionType.Copy
MUL = mybir.AluOpType.mult
ADD = mybir.AluOpType.add

N_CORES = 8
P = 128          # partitions
M = 4096         # input row length
ROWS = 512       # rows per core
NT = ROWS // P   # row tiles per core
WU = 1024        # panel width (input cols)
NPAN = M // WU   # panels per tile
NCHUNK = 512     # psum chunk (one bank of fp32)

NDVE = int(os.environ.get("DWT_NDVE", "6"))     # panels (of NT*NPAN=16) on DVE
REPS = int(os.environ.get("DWT_REPS", "1"))     # benchmark-only in-kernel loop
IO_ONLY = bool(int(os.environ.get("DWT_IO_ONLY", "0")))
EVICT_ENG = os.environ.get("DWT_EVICT", "scalar")  # psum eviction engine
CVT_ENG = os.environ.get("DWT_CVT", "scalar")      # fp16 conversion engine


def _dve_panels():
    # Spread DVE panels across tiles and halves.
    order = [(t, p) for p in (1, 3, 0, 2) for t in range(NT)]
    return set(order[:NDVE])


def build_nc(dve_set, wavelet_vals=None):
    if wavelet_vals is None:
        # DB4 defaults (reference.setup_inputs uses these); kernel() always
        # rebuilds with the actual runtime wavelet on first call.
        wavelet_vals = np.array([-0.010597401784997278, 0.032883011666982945,
                                 0.030841381835986965, -0.18703481171888114,
                                 -0.02798376941698385, 0.6308807679295904,
                                 0.7148465705525415, 0.23037781330885523],
                                dtype=np.float64)
    wv64 = [float(v) for v in np.asarray(wavelet_vals, dtype=np.float64)]
    nc = bacc.Bacc()
    det = nc.declare_dram_parameter("details", [ROWS, M], F32, isOutput=False)
    app = nc.declare_dram_parameter("approximation", [ROWS, M], F32, isOutput=False)
    wav = nc.declare_dram_parameter("wavelet", [8], F32, isOutput=False)
    res = nc.declare_dram_parameter("result", [ROWS, 2 * M], F32, isOutput=True)
    ident = nc.inline_tensor(np.eye(P, dtype=np.float32), "ident")

    with tile.TileContext(nc) as tc:
        with (
            tc.tile_pool(name="const", bufs=1) as constp,
            tc.tile_pool(name="ine", bufs=2) as inp,
            tc.tile_pool(name="oute", bufs=3) as outp,
            tc.tile_pool(name="cvt", bufs=8) as cvtp,
            tc.tile_pool(name="acc", bufs=4) as accp,
            tc.tile_pool(name="psum", bufs=8, space="PSUM") as psump,
        ):
            # ---- coefficients as compile-time immediates: the scalar op
            # then lowers to TensorScalar (not TensorScalarPtr), whose 2x/4x
            # DVE uops exist; the Ptr variant runs at 1x. The NEFF is
            # specialized on the runtime wavelet values by kernel().
            # Token read keeps the "wavelet" ExternalInput alive in the NEFF.
            wv = constp.tile([1, 8], F32)
            nc.sync.dma_start(wv[:], wav[None, :])

            def coeff(x, pi, s):
                # x: 0 = details, 1 = approximation; pi: 0 = even, 1 = odd
                if x == 0:
                    return wv64[7 - 2 * s] if pi == 0 else wv64[6 - 2 * s]
                if pi == 0:
                    return wv64[2 * s]
                return -wv64[2 * s + 1]

            # ---- PE weights: c * I for each (input, parity, tap); F32R-typed
            # so the BIR verifier accepts them as f32r matmul operands.
            it = constp.tile([P, P], F32)
            nc.sync.dma_start(it[:], ident[:, :])
            w16 = constp.tile([P, 16 * P], F32R)

            def wslice(x, pi, s):
                j = (x * 2 + pi) * 4 + s
                return w16[:, j * P : (j + 1) * P]

            for x in range(2):
                for pi in range(2):
                    for s in range(4):
                        nc.vector.tensor_scalar(
                            wslice(x, pi, s), it[:], coeff(x, pi, s), None, MUL
                        )

            taps = [(x, s) for x in range(2) for s in range(4)]
            cvt_op = nc.scalar.copy if CVT_ENG == "scalar" else nc.vector.tensor_copy
            evict_op = nc.scalar.copy if EVICT_ENG == "scalar" else nc.vector.tensor_copy

            def body(_i=None):
              for t in range(NT):
                r0 = t * P
                # F32R-typed so PE can consume windows directly; every
                # non-PE reader bitcasts back to F32 (same bits).
                dext = inp.tile([P, M + 3], F32R, tag="dext")
                nc.sync.dma_start(dext[:, 3 : M + 3], det[r0 : r0 + P, :].bitcast(F32R))
                aext = inp.tile([P, M + 3], F32R, tag="aext")
                nc.sync.dma_start(aext[:, 3 : M + 3], app[r0 : r0 + P, :].bitcast(F32R))
                nc.vector.tensor_copy(dext[:, 0:3], dext[:, M : M + 3])
                nc.vector.tensor_copy(aext[:, 0:3], aext[:, M : M + 3])
                ext = [dext, aext]

                def xf(x, a, b):
                    return ext[x][:, a:b].bitcast(F32)

                for h in range(2):  # two output half-tiles of M cols each
                    oh = outp.tile([P, M], F32, tag="out")
                    for p in range(h * (NPAN // 2), (h + 1) * (NPAN // 2)):
                        c0 = p * WU
                        if IO_ONLY:
                            for pi in range(2):
                                base = 2 * c0 - h * M + pi
                                nc.scalar.copy(
                                    oh[:, base : min(base + 2 * WU, M) : 2],
                                    xf(0, 3 + c0, 3 + c0 + WU),
                                )
                            continue
                        if (t, p) in dve_set:
                            # fp16 aligned copies: bufE = ext[c0:...], bufO = ext[c0+1:...]
                            bE, bO = [], []
                            for x in range(2):
                                be = cvtp.tile([P, WU + 2], F16, tag="cvt")
                                cvt_op(be[:], xf(x, c0, c0 + WU + 2))
                                bo = cvtp.tile([P, WU + 2], F16, tag="cvt")
                                cvt_op(bo[:], xf(x, c0 + 1, c0 + WU + 3))
                                bE.append(be)
                                bO.append(bo)

                            def win(x, s):
                                # tap window = ext[3-s+c0 : 3-s+c0+WU]; bufE holds
                                # ext[c0:...], bufO holds ext[c0+1:...] — offsets
                                # 3-s resp. 2-s are always even (4B-aligned fp16).
                                if s in (1, 3):
                                    return bE[x][:, 3 - s : 3 - s + WU]
                                return bO[x][:, 2 - s : 2 - s + WU]

                            for pi in range(2):
                                base = 2 * c0 - h * M + pi
                                oview = oh[:, base : min(base + 2 * WU, M) : 2]
                                chain = [(0, 3), (0, 1), (0, 2), (0, 0),
                                         (1, 3), (1, 1), (1, 2)]
                                acc = accp.tile([P, WU], F16, tag="acc")
                                x0, s0 = chain[0]
                                nc.vector.tensor_scalar(
                                    acc[:], win(x0, s0), coeff(x0, pi, s0), None, MUL
                                )
                                for x, s in chain[1:]:
                                    nc.vector.scalar_tensor_tensor(
                                        acc[:], win(x, s), coeff(x, pi, s),
                                        acc[:], MUL, ADD,
                                    )
                                nc.vector.scalar_tensor_tensor(
                                    oview, win(1, 0), coeff(1, pi, 0),
                                    acc[:], MUL, ADD,
                                )
                        else:  # PE panel: f32r identity-scaled matmuls
                            for pi in range(2):
                                ccs = list(range(c0, c0 + WU, NCHUNK))
                                pss = [
                                    psump.tile([P, NCHUNK], F32, tag="ps",
                                               name=f"ps_{t}_{pi}_{p}_{ci}")
                                    for ci in range(len(ccs))
                                ]
                                for j, (x, s) in enumerate(taps):
                                    w = wslice(x, pi, s)
                                    for ci, cc in enumerate(ccs):
                                        rhs = ext[x][:, 3 - s + cc : 3 - s + cc + NCHUNK]
                                        nc.tensor.matmul(
                                            pss[ci][:], w, rhs,
                                            start=(j == 0), stop=(j == len(taps) - 1),
                                        )
                                for ci, cc in enumerate(ccs):
                                    evb = 2 * cc - h * M + pi
                                    evict_op(
                                        oh[:, evb : min(evb + 2 * NCHUNK, M) : 2], pss[ci][:]
                                    )
                    nc.sync.dma_start(res[r0 : r0 + P, h * M : (h + 1) * M], oh[:])

            if REPS == 1:
                body()
            else:
                with tc.For_i(0, REPS, 1) as _rv:
                    body(_rv)
    nc.finalize()
    return nc


_CACHE = {}


def _get_nc(wavelet):
    key = wavelet.tobytes()
    if _CACHE.get("key") != key:
        _CACHE["nc"] = build_nc(_dve_panels(), wavelet)
        _CACHE["key"] = key
    return _CACHE["nc"]


def kernel(details, approximation, wavelet):
    details = np.ascontiguousarray(np.asarray(details, dtype=np.float32))
    approximation = np.ascontiguousarray(np.asarray(approximation, dtype=np.float32))
    wavelet = np.ascontiguousarray(np.asarray(wavelet, dtype=np.float32))
    assert details.shape == (N_CORES * ROWS, M) and approximation.shape == details.shape
    assert wavelet.shape == (8,)

    in_maps = [
        {
            "details": details[c * ROWS : (c + 1) * ROWS],
            "approximation": approximation[c * ROWS : (c + 1) * ROWS],
            "wavelet": wavelet,
        }
        for c in range(N_CORES)
    ]
    trace = bool(int(os.environ.get("DWT_TRACE", "0")))
    r = run_bass_kernel_spmd(_get_nc(wavelet), in_maps, list(range(N_CORES)), trace=trace)
    _CACHE["last_results"] = r
    return np.concatenate([r.results[c]["result"] for c in range(N_CORES)], axis=0)


# revision 3
# speedup vs baseline: 1.4681x; 1.3131x over previous
"""Inverse DWT (BackwardTransformLayer) Trainium2 Bass kernel — v2.

Math (polyphase form of the zero-interleaved circular FFT convolution):
  out[r, 2p+pi] = sum_{s=0..3} cD[pi,s]*D[r,(p-s)%M] + cA[pi,s]*A[r,(p-s)%M]
  cD[0,s] = w[7-2s]   cD[1,s] = w[6-2s]   cA[0,s] = w[2s]   cA[1,s] = -w[2s+1]

Sharding: data-parallel over rows; 512 rows per core on 8 NeuronCores.

Measured 126965 ns (REPS-slope method; all-fp32 predecessor: 235940 ns by
the same method), rel err 1.3e-3 vs the 2e-2 gate. The wins, in order:
  - PE panels (10 of 16): f32r (tf32-like) matmuls of identity-scaled
    weights reading the fp32 input tiles directly via bitcast — 1 cyc/row
    instead of fp32's 4, no conversion pass. Tiles are F32R-typed and DMA'd
    via a bitcast source AP because the BIR verifier requires f32r matmul
    operands to be f32r-typed ("rounded"); every other reader bitcasts
    back to F32.
  - Engine rebalance: 6 DVE panels / 10 PE panels (the old 21/11 split left
    DVE as a ~190us critical path). DVE chains are fp16
    scalar_tensor_tensor MACs; note STT has no fast DVE uop — it runs 1x
    regardless of dtype (confirmed in CoreSim cost model AND by HW timing).
    Coefficients are compile-time immediates: the NEFF is specialized on
    the runtime wavelet values by kernel() (works for any wavelet).
  - ScalarE produces two fp16 copies per DVE panel (bufE at ext[c0], bufO
    at ext[c0+1], windows at even offsets) and evicts PE PSUM stride-2.
Floor: ~93-100us/core (33.5 MB HBM traffic at ~360 GB/s); kernel runs at
~127us, i.e. ~35% above floor due to compute/scheduling overlap losses.
Tried and measured SLOWER, do not redo without new evidence:
  - tensor_scalar(4x-claimed)+tensor_tensor tree chains, input-DMA halving,
    prefix-from-HBM, INP/OUT_BUFS 3/3: CoreSim said 112.8us, HW said
    142.0us — the cost model's fast-mode table for TensorScalarPtr
    overestimates real HW.
  - Parity-granular DVE/PE assignment: 126-129us in sim, never beat
    whole-panel granularity.
"""

import os
import sys

import numpy as np

for _p in ("/opt/trn_rl_repo", "/root/.axon_site/_ro/trn_rl_repo"):
    if os.path.isdir(_p) and _p not in sys.path:
        sys.path.append(_p)

import concourse.bass as bass  # noqa: E402
import concourse.tile as tile  # noqa: E402
from concourse import bacc, mybir  # noqa: E402
from concourse.bass_utils import run_bass_kernel_spmd  # noqa: E402

F32 = mybir.dt.float32
F32R = mybir.dt.float32r
F16 = mybir.dt.float16
COPY = mybir.ActivationFunctionType.Copy
MUL = mybir.AluOpType.mult
ADD = mybir.AluOpType.add

N_CORES = 8
P = 128          # partitions
M = 4096         # input row length
ROWS = 512       # rows per core
NT = ROWS // P   # row tiles per core
WU = 1024        # panel width (input cols)
NPAN = M // WU   # panels per tile
NCHUNK = 512     # psum chunk (one bank of fp32)

NDVE = int(os.environ.get("DWT_NDVE", "6"))     # panels (of NT*NPAN=16) on DVE
REPS = int(os.environ.get("DWT_REPS", "1"))     # benchmark-only in-kernel loop
IO_ONLY = bool(int(os.environ.get("DWT_IO_ONLY", "0")))
EVICT_ENG = os.environ.get("DWT_EVICT", "scalar")  # psum eviction engine
CVT_ENG = os.environ.get("DWT_CVT", "scalar")      # fp16 conversion engine


def _dve_panels():
    # Spread DVE panels across tiles and halves.
    order = [(t, p) for p in (1, 3, 0, 2) for t in range(NT)]
    return set(order[:NDVE])


def build_nc(dve_set, wavelet_vals=None):
    if wavelet_vals is None:
        # DB4 defaults (reference.setup_inputs uses these); kernel() always
        # rebuilds with the actual runtime wavelet on first call.
        wavelet_vals = np.array([-0.010597401784997278, 0.032883011666982945,
                                 0.030841381835986965, -0.18703481171888114,
                                 -0.02798376941698385, 0.6308807679295904,
                                 0.7148465705525415, 0.23037781330885523],
                                dtype=np.float64)
    wv64 = [float(v) for v in np.asarray(wavelet_vals, dtype=np.float64)]
    nc = bacc.Bacc()
    det = nc.declare_dram_parameter("details", [ROWS, M], F32, isOutput=False)
    app = nc.declare_dram_parameter("approximation", [ROWS, M], F32, isOutput=False)
    wav = nc.declare_dram_parameter("wavelet", [8], F32, isOutput=False)
    res = nc.declare_dram_parameter("result", [ROWS, 2 * M], F32, isOutput=True)
    ident = nc.inline_tensor(np.eye(P, dtype=np.float32), "ident")

    with tile.TileContext(nc) as tc:
        with (
            tc.tile_pool(name="const", bufs=1) as constp,
            tc.tile_pool(name="ine", bufs=2) as inp,
            tc.tile_pool(name="oute", bufs=3) as outp,
            tc.tile_pool(name="cvt", bufs=8) as cvtp,
            tc.tile_pool(name="acc", bufs=4) as accp,
            tc.tile_pool(name="psum", bufs=8, space="PSUM") as psump,
        ):
            # ---- coefficients as compile-time immediates: the scalar op
            # then lowers to TensorScalar (not TensorScalarPtr), whose 2x/4x
            # DVE uops exist; the Ptr variant runs at 1x. The NEFF is
            # specialized on the runtime wavelet values by kernel().
            # Token read keeps the "wavelet" ExternalInput alive in the NEFF.
            wv = constp.tile([1, 8], F32)
            nc.sync.dma_start(wv[:], wav[None, :])

            def coeff(x, pi, s):
                # x: 0 = details, 1 = approximation; pi: 0 = even, 1 = odd
                if x == 0:
                    return wv64[7 - 2 * s] if pi == 0 else wv64[6 - 2 * s]
                if pi == 0:
                    return wv64[2 * s]
                return -wv64[2 * s + 1]

            # ---- PE weights: c * I for each (input, parity, tap); F32R-typed
            # so the BIR verifier accepts them as f32r matmul operands.
            it = constp.tile([P, P], F32)
            nc.sync.dma_start(it[:], ident[:, :])
            w16 = constp.tile([P, 16 * P], F32R)

            def wslice(x, pi, s):
                j = (x * 2 + pi) * 4 + s
                return w16[:, j * P : (j + 1) * P]

            for x in range(2):
                for pi in range(2):
                    for s in range(4):
                        nc.vector.tensor_scalar(
                            wslice(x, pi, s), it[:], coeff(x, pi, s), None, MUL
                        )

            taps = [(x, s) for x in range(2) for s in range(4)]
            cvt_op = nc.scalar.copy if CVT_ENG == "scalar" else nc.vector.tensor_copy
            evict_op = nc.scalar.copy if EVICT_ENG == "scalar" else nc.vector.tensor_copy

            def body(_i=None):
              for t in range(NT):
                r0 = t * P
                # F32R-typed so PE can consume windows directly; every
                # non-PE reader bitcasts back to F32 (same bits).
                dext = inp.tile([P, M + 3], F32R, tag="dext")
                nc.sync.dma_start(dext[:, 3 : M + 3], det[r0 : r0 + P, :].bitcast(F32R))
                aext = inp.tile([P, M + 3], F32R, tag="aext")
                nc.sync.dma_start(aext[:, 3 : M + 3], app[r0 : r0 + P, :].bitcast(F32R))
                nc.vector.tensor_copy(dext[:, 0:3], dext[:, M : M + 3])
                nc.vector.tensor_copy(aext[:, 0:3], aext[:, M : M + 3])
                ext = [dext, aext]

                def xf(x, a, b):
                    return ext[x][:, a:b].bitcast(F32)

                for h in range(2):  # two output half-tiles of M cols each
                    oh = outp.tile([P, M], F32, tag="out")
                    for p in range(h * (NPAN // 2), (h + 1) * (NPAN // 2)):
                        c0 = p * WU
                        if IO_ONLY:
                            for pi in range(2):
                                base = 2 * c0 - h * M + pi
                                nc.scalar.copy(
                                    oh[:, base : min(base + 2 * WU, M) : 2],
                                    xf(0, 3 + c0, 3 + c0 + WU),
                                )
                            continue
                        if (t, p) in dve_set:
                            # fp16 aligned copies: bufE = ext[c0:...], bufO = ext[c0+1:...]
                            bE, bO = [], []
                            for x in range(2):
                                be = cvtp.tile([P, WU + 2], F16, tag="cvt")
                                cvt_op(be[:], xf(x, c0, c0 + WU + 2))
                                bo = cvtp.tile([P, WU + 2], F16, tag="cvt")
                                cvt_op(bo[:], xf(x, c0 + 1, c0 + WU + 3))
                                bE.append(be)
                                bO.append(bo)

                            def win(x, s):
                                # tap window = ext[3-s+c0 : 3-s+c0+WU]; bufE holds
                                # ext[c0:...], bufO holds ext[c0+1:...] — offsets
                                # 3-s resp. 2-s are always even (4B-aligned fp16).
                                if s in (1, 3):
                                    return bE[x][:, 3 - s : 3 - s + WU]
                                return bO[x][:, 2 - s : 2 - s + WU]

                            for pi in range(2):
                                base = 2 * c0 - h * M + pi
                                oview = oh[:, base : min(base + 2 * WU, M) : 2]
                                chain = [(0, 3), (0, 1), (0, 2), (0, 0),
                                         (1, 3), (1, 1), (1, 2)]
                                acc = accp.tile([P, WU], F16, tag="acc")
                                x0, s0 = chain[0]
                                nc.vector.tensor_scalar(
                                    acc[:], win(x0, s0), coeff(x0, pi, s0), None, MUL
                                )
                                for x, s in chain[1:]:
                                    nc.vector.scalar_tensor_tensor(
                                        acc[:], win(x, s), coeff(x, pi, s),
                                        acc[:], MUL, ADD,
                                    )
                                nc.vector.scalar_tensor_tensor(
                                    oview, win(1, 0), coeff(1, pi, 0),
                                    acc[:], MUL, ADD,
                                )
                        else:  # PE panel: f32r identity-scaled matmuls
                            for pi in range(2):
                                ccs = list(range(c0, c0 + WU, NCHUNK))
                                pss = [
                                    psump.tile([P, NCHUNK], F32, tag="ps",
                                               name=f"ps_{t}_{pi}_{p}_{ci}")
                                    for ci in range(len(ccs))
                                ]
                                for j, (x, s) in enumerate(taps):
                                    w = wslice(x, pi, s)
                                    for ci, cc in enumerate(ccs):
                                        rhs = ext[x][:, 3 - s + cc : 3 - s + cc + NCHUNK]
                                        nc.tensor.matmul(
                                            pss[ci][:], w, rhs,
                                            start=(j == 0), stop=(j == len(taps) - 1),
                                        )
                                for ci, cc in enumerate(ccs):
                                    evb = 2 * cc - h * M + pi
                                    evict_op(
                                        oh[:, evb : min(evb + 2 * NCHUNK, M) : 2], pss[ci][:]
                                    )
                    nc.sync.dma_start(res[r0 : r0 + P, h * M : (h + 1) * M], oh[:])

            if REPS == 1:
                body()
            else:
                with tc.For_i(0, REPS, 1) as _rv:
                    body(_rv)
    nc.finalize()
    return nc


_CACHE = {}


def _get_nc(wavelet):
    key = wavelet.tobytes()
    if _CACHE.get("key") != key:
        _CACHE["nc"] = build_nc(_dve_panels(), wavelet)
        _CACHE["key"] = key
    return _CACHE["nc"]


def kernel(details, approximation, wavelet):
    details = np.ascontiguousarray(np.asarray(details, dtype=np.float32))
    approximation = np.ascontiguousarray(np.asarray(approximation, dtype=np.float32))
    wavelet = np.ascontiguousarray(np.asarray(wavelet, dtype=np.float32))
    assert details.shape == (N_CORES * ROWS, M) and approximation.shape == details.shape
    assert wavelet.shape == (8,)

    in_maps = [
        {
            "details": details[c * ROWS : (c + 1) * ROWS],
            "approximation": approximation[c * ROWS : (c + 1) * ROWS],
            "wavelet": wavelet,
        }
        for c in range(N_CORES)
    ]
    trace = bool(int(os.environ.get("DWT_TRACE", "0")))
    r = run_bass_kernel_spmd(_get_nc(wavelet), in_maps, list(range(N_CORES)), trace=trace)
    _CACHE["last_results"] = r
    return np.concatenate([r.results[c]["result"] for c in range(N_CORES)], axis=0)
